# revision 78
# baseline (speedup 1.0000x reference)
"""Causal self-attention (B=2, T=2048, D=1024, H=16) on 8 trn2 cores.

Sharding: core c -> (batch b = c//4, head-group g = c%4, 4 heads each).
Data-parallel over B, tensor-parallel over heads; the output projection is
computed as per-core partials (each core owns 256 of the 1024 contraction
dims) summed on the host.

Per-core layout choices (all chosen so no on-device transposes are needed):
  - x is shipped feature-major x^T [D, T] (bf16, host-prepared)
  - q^T, k^T are computed feature-major [n, t] (lhsT = W^T, rhs = x^T)
  - v is computed token-major [t, e] (lhsT = x^T, rhs = W_v^T) and stored
    with a ones-column appended, so the P@V matmul emits softmax sums in
    PSUM row 64 for free
  - scores are computed transposed st[j, i] = k q^T; softmax runs without
    max-subtraction (scores are O(1) here), sums via the ones-column
  - causality: j-tile loop is bounded by the i-block, plus one triangular
    128x128 mask multiply on diagonal blocks
"""

import numpy as np
import ml_dtypes

import concourse.bass as bass
import concourse.bacc as bacc
import concourse.mybir as mybir
import concourse.tile as tile
from concourse.bass_utils import run_bass_kernel_spmd

BF16 = mybir.dt.bfloat16
F32 = mybir.dt.float32

B, T, D, H, HD = 2, 2048, 1024, 16, 64
N_CORES = 8
SCALE = HD ** -0.5  # 0.125


def _emit(nc, tc, xT_d, wqk_d, wv_d, wp_d, bqk_d, out_d):
    mult = mybir.AluOpType.mult
    Exp = mybir.ActivationFunctionType.Exp

    with (
        tc.tile_pool(name="const", bufs=1) as cpool,
        tc.tile_pool(name="work", bufs=12) as wpool,
        tc.tile_pool(name="norm", bufs=10) as npool,
        tc.tile_pool(name="stage", bufs=8) as spool,
        tc.tile_pool(name="ps_st", bufs=3, space="PSUM") as ps_st,
        tc.tile_pool(name="ps_pv", bufs=1, space="PSUM") as ps_pv,
    ):
        # ---- constants / persistent tensors -----------------------------
        mask_sb = cpool.tile([128, 128], BF16, name="mask_sb")
        nc.vector.memset(mask_sb[:], 1.0)
        # keep where (i - j) >= 0  (j = partition, i = free)
        nc.gpsimd.affine_select(
            out=mask_sb[:], in_=mask_sb[:],
            compare_op=mybir.AluOpType.is_ge, fill=0.0,
            base=0, channel_multiplier=-1, pattern=[[1, 128]],
        )

        # x^T streamed in column chunks (512 / 512 / 1024); the first wave is
        # interleaved per k-tile with the matching w_qk slab so the first
        # accumulation matmul starts after two small DMAs, not the whole load
        wqk_sb = cpool.tile([128, 8, 512], BF16, name="wqk_sb")
        wqr = wqk_d.ap().rearrange("(k p) n -> p k n", p=128)
        xt_sb = cpool.tile([128, 8, T], BF16, name="xt_sb")
        xr = xT_d.ap().rearrange("(k p) t -> p k t", p=128)
        for k in range(8):
            nc.sync.dma_start(out=wqk_sb[:, k, :], in_=wqr[:, k, :])
            nc.sync.dma_start(out=xt_sb[:, k, 0:512], in_=xr[:, k, 0:512])
        bqk_sb = cpool.tile([128, 4], F32, name="bqk_sb")
        nc.sync.dma_start(out=bqk_sb[:], in_=bqk_d[:])
        wv_sb = cpool.tile([128, 8, 256], BF16, name="wv_sb")
        nc.sync.dma_start(
            out=wv_sb[:], in_=wv_d.ap().rearrange("(k p) n -> p k n", p=128))
        for c0, c1 in ((512, 1024), (1024, 2048)):
            for k in range(8):
                nc.sync.dma_start(out=xt_sb[:, k, c0:c1],
                                  in_=xr[:, k, c0:c1])

        wp_sb = cpool.tile([128, 2, 1024], BF16, name="wp_sb")
        nc.sync.dma_start(
            out=wp_sb[:], in_=wp_d.ap().rearrange("(k p) n -> p k n", p=128))

        # sums staging for the batched reciprocal: rows 0 / 64 hold the two
        # heads' softmax sums; other rows stay 1.0 so reciprocal is finite
        srow_sb = cpool.tile([65, 512], F32, name="srow_sb")
        nc.vector.memset(srow_sb[:], 1.0)

        qkT_sb = cpool.tile([128, 4, T], BF16, name="qkT_sb")
        vaug_sb = cpool.tile([128, 4, 16, 65], BF16, name="vaug_sb")
        nc.vector.memset(vaug_sb[:], 1.0)  # ones column (col 64) survives
        outT_sb = cpool.tile([128, 2, T], BF16, name="outT_sb")

        # ---- phase A: q^T / k^T projection  (feature-major) -------------
        # m-tile: 0 = q heads 0-1, 1 = q heads 2-3, 2 = k heads 0-1, 3 = k 2-3
        def qk_unit(m, tb):
            ps = ps_st.tile([128, 512], F32, name=f"psqk{m}{tb}", tag="st")
            for k in range(8):
                nc.tensor.matmul(
                    ps[:],
                    lhsT=wqk_sb[:, k, 128 * m:128 * m + 128],
                    rhs=xt_sb[:, k, 512 * tb:512 * tb + 512],
                    start=(k == 0), stop=(k == 7),
                )
            # ScalarE eviction keeps the DVE queue (psum evictions,
            # reciprocals) from gating the shared psum slots
            nc.scalar.activation(
                out=qkT_sb[:, m, 512 * tb:512 * tb + 512],
                in_=ps[:], func=mybir.ActivationFunctionType.Identity,
                bias=bqk_sb[:, m:m + 1],
            )

        # ---- phase B: v projection (token-major, no bias) ---------------
        def emit_v(jt):
            psv = ps_st.tile([128, 256], F32, name=f"psv{jt}", tag="st")
            for k in range(8):
                nc.tensor.matmul(
                    psv[:],
                    lhsT=xt_sb[:, k, 128 * jt:128 * jt + 128],
                    rhs=wv_sb[:, k, :],
                    start=(k == 0), stop=(k == 7),
                )
            nc.vector.tensor_copy(
                out=vaug_sb[:, :, jt, 0:64],
                in_=psv[:].rearrange("p (h e) -> p h e", h=4),
            )

        # ---- phase C: attention for head pair hp (heads 2hp, 2hp+1) -----
        # Normalization (reciprocal + broadcast + scale) of block N is
        # emitted at the start of block N+1 so the expensive DVE reciprocal
        # overlaps the next block's matmuls (pv pool is double-buffered).
        pending = []

        def emit_norm(upv, hp, it):
            # both heads' sums into srow rows 0 / 64, one batched reciprocal
            # (cost scales with free size, not rows), then per-head broadcast.
            # partition_broadcast requires base partition 0, so head1's recip
            # row is first copied down to a base-0 tile.
            nc.vector.tensor_copy(out=srow_sb[0:1, :], in_=upv[0][64:65, :])
            nc.vector.tensor_copy(out=srow_sb[64:65, :], in_=upv[1][64:65, :])
            rec = npool.tile([65, 512], F32, name=f"rec{hp}{it}", tag="rec")
            nc.vector.reciprocal(out=rec[:], in_=srow_sb[:])
            recb = npool.tile([1, 512], F32, name=f"recb{hp}{it}", tag="recb")
            nc.vector.tensor_copy(out=recb[:], in_=rec[64:65, :])
            for h in range(2):
                bc = npool.tile([64, 512], F32, name=f"bc{hp}{it}{h}",
                                tag="bc")
                nc.gpsimd.partition_broadcast(
                    bc[:], rec[0:1, :] if h == 0 else recb[:])
                osl = outT_sb[64 * h:64 * h + 64, hp,
                              512 * it:512 * it + 512]
                # v-bias is folded into the host-side output bias
                nc.vector.tensor_tensor(
                    out=osl, in0=upv[h][0:64, :], in1=bc[:], op=mult)

        def emit_attn(hp, its, fillers=()):
            # fillers: small PE-only work units (qk/v/proj psum chains)
            # interleaved between attention j-tiles so the PE has independent
            # work while ScalarE exponentiates
            fillers = list(fillers)
            njt_total = sum(4 * (it + 1) for it in its)
            space = max(1, njt_total // (len(fillers) + 1)) if fillers else 0
            jt_count = 0
            for it in its:
                pv = []
                for h in range(2):
                    t_ = ps_pv.tile([65, 512], F32, name=f"pv{hp}{it}{h}",
                                    tag=f"pv{h}")
                    pv.append(t_)
                njt = 4 * (it + 1)
                for jt in range(njt):
                    diag = jt - 4 * it  # >= 0 on/near the diagonal
                    off = 128 * diag if diag >= 0 else 0
                    width = 512 - off
                    st = ps_st.tile([128, 1024], F32, name=f"st{hp}{it}{jt}",
                                    tag="st")
                    for h in range(2):
                        # explicit row-group tile_position: the two heads'
                        # K=64 matmuls run concurrently in disjoint array rows
                        nc.tensor.matmul(
                            st[:, 512 * h + off:512 * h + 512],
                            lhsT=qkT_sb[64 * h:64 * h + 64, 2 + hp,
                                        128 * jt:128 * jt + 128],
                            rhs=qkT_sb[64 * h:64 * h + 64, hp,
                                       512 * it + off:512 * it + 512],
                            start=True, stop=True,
                            tile_position=(64 * h, 0),
                        )
                    ex = wpool.tile([128, 1024], BF16, name=f"ex{hp}{it}{jt}",
                                    tag="ex")
                    st_v = st[:].rearrange("p (h i) -> p h i", h=2)
                    ex_v = ex[:].rearrange("p (h i) -> p h i", h=2)
                    nc.scalar.activation(
                        out=ex_v[:, :, off:512], in_=st_v[:, :, off:512],
                        func=Exp, scale=SCALE,
                    )
                    if diag >= 0:
                        for h in range(2):
                            sl = ex[:, 512 * h + off:512 * h + off + 128]
                            nc.vector.tensor_tensor(
                                out=sl, in0=sl, in1=mask_sb[:], op=mult)
                    for h in range(2):
                        nc.tensor.matmul(
                            pv[h][:, off:512],
                            lhsT=vaug_sb[:, 2 * hp + h, jt, :],
                            rhs=ex[:, 512 * h + off:512 * h + 512],
                            start=(jt == 0), stop=(jt == njt - 1),
                        )
                    jt_count += 1
                    if jt == min(1, njt - 1) and pending:
                        # normalize the previous block mid-loop: the first
                        # mask-multiplies of this block are already queued on
                        # DVE ahead of the expensive reciprocal
                        emit_norm(*pending.pop(0))
                    if fillers and jt_count % space == 0:
                        fillers.pop(0)()
                # evict the PV accumulators (+ sums row) to SBUF right away:
                # frees the psum banks after one quick DVE copy instead of
                # after the whole normalization chain
                upv = []
                for h in range(2):
                    u = npool.tile([65, 512], F32, name=f"upv{hp}{it}{h}",
                                   tag=f"upv{h}", bufs=2)
                    nc.vector.tensor_copy(out=u[:], in_=pv[h][:])
                    upv.append(u)
                pending.append((upv, hp, it))
            for f in fillers:
                f()

        # ---- phase D: output projection partials (bf16, summed on host) --
        def proj_unit(tt):
            for ch in range(2):
                psp = ps_st.tile([128, 512], F32, name=f"psp{tt}{ch}",
                                 tag="st")
                for ek in range(2):
                    nc.tensor.matmul(
                        psp[:],
                        lhsT=outT_sb[:, ek, 128 * tt:128 * tt + 128],
                        rhs=wp_sb[:, ek, 512 * ch:512 * ch + 512],
                        start=(ek == 0), stop=(ek == 1),
                    )
                stg = spool.tile([128, 512], BF16, name=f"stg{tt}{ch}",
                                 tag="stage")
                if tt >= 12 and ch == 0:
                    # tail blocks: DVE is idle here — split the eviction
                    # queues so neither engine serializes the finish
                    nc.vector.tensor_copy(out=stg[:], in_=psp[:])
                else:
                    # ScalarE copy: keeps psum eviction off the DVE queue
                    nc.scalar.copy(out=stg[:], in_=psp[:])
                nc.sync.dma_start(
                    out=out_d.ap()[128 * tt:128 * tt + 128,
                                   512 * ch:512 * ch + 512],
                    in_=stg[:])

        # Interleaved emission: attention (exp on ScalarE is the long pole)
        # starts as soon as its first q/k/v tiles are projected; remaining
        # projection work is spread through the attention loops as fillers.
        qk_unit(0, 0)
        qk_unit(2, 0)
        for jt in range(4):
            emit_v(jt)
        emit_attn(0, [0])
        qk_unit(0, 1)
        qk_unit(2, 1)
        for jt in range(4, 8):
            emit_v(jt)
        emit_attn(0, [1])
        for tb in range(2, 4):
            qk_unit(0, tb)
            qk_unit(2, tb)
        for jt in range(8, 16):
            emit_v(jt)
        emit_attn(0, [2, 3])
        for tb in range(2):
            qk_unit(1, tb)
            qk_unit(3, tb)
        emit_attn(1, [0, 1])
        for tb in range(2, 4):
            qk_unit(1, tb)
            qk_unit(3, tb)
        for tt in range(4):              # t-block 0 (norms hp0/hp1 done)
            proj_unit(tt)
        emit_attn(1, [2, 3])
        emit_norm(*pending.pop(0))       # last block's normalization
        for tt in range(4, 16):
            proj_unit(tt)


def build_program():
    nc = bacc.Bacc("TRN2", target_bir_lowering=False, debug=False,
                   num_devices=N_CORES)
    xT_d = nc.dram_tensor("xT", [D, T], BF16, kind="ExternalInput")
    wqk_d = nc.dram_tensor("wqk", [D, 512], BF16, kind="ExternalInput")
    wv_d = nc.dram_tensor("wv", [D, 256], BF16, kind="ExternalInput")
    wp_d = nc.dram_tensor("wp", [256, D], BF16, kind="ExternalInput")
    bqk_d = nc.dram_tensor("bqk", [128, 4], F32, kind="ExternalInput")
    out_d = nc.dram_tensor("out", [T, D], BF16, kind="ExternalOutput")
    with tile.TileContext(nc) as tc:
        _emit(nc, tc, xT_d, wqk_d, wv_d, wp_d, bqk_d, out_d)
    nc.compile()
    return nc


def make_in_maps(x, w_qkv, b_qkv, w_proj):
    bf = ml_dtypes.bfloat16
    in_maps = []
    for c in range(N_CORES):
        b, g = c // 4, c % 4
        qs, ks, vs = 256 * g, 1024 + 256 * g, 2048 + 256 * g
        qb = b_qkv[qs:qs + 256]
        kb = b_qkv[ks:ks + 256]
        vb = b_qkv[vs:vs + 256]
        in_maps.append({
            "xT": np.ascontiguousarray(x[b].T).astype(bf),
            "wqk": np.ascontiguousarray(
                np.concatenate([w_qkv[qs:qs + 256], w_qkv[ks:ks + 256]], 0).T
            ).astype(bf),
            "wv": np.ascontiguousarray(w_qkv[vs:vs + 256].T).astype(bf),
            "wp": np.ascontiguousarray(w_proj[:, 256 * g:256 * g + 256].T
                                       ).astype(bf),
            "bqk": np.ascontiguousarray(
                np.stack([qb[:128], qb[128:], kb[:128], kb[128:]], 1)
            ).astype(np.float32),
        })
    return in_maps


def assemble(results, w_proj, b_qkv, b_proj):
    out = np.zeros((B, T, D), np.float32)
    for c in range(N_CORES):
        out[c // 4] += results[c]["out"].astype(np.float32)
    # v-bias passes through attention unchanged (rows of P sum to 1), so its
    # contribution to the output is the constant w_proj @ b_v
    bias = b_proj.astype(np.float32) + w_proj.astype(np.float32) @ \
        b_qkv[2048:3072].astype(np.float32)
    out += bias[None, None, :]
    return out


_NC_CACHE = []


def run(x, w_qkv, b_qkv, w_proj, b_proj, trace=False):
    if not _NC_CACHE:
        _NC_CACHE.append(build_program())
    nc = _NC_CACHE[0]
    in_maps = make_in_maps(np.asarray(x, np.float32), np.asarray(w_qkv, np.float32),
                           np.asarray(b_qkv, np.float32), np.asarray(w_proj, np.float32))
    res = run_bass_kernel_spmd(nc, in_maps, core_ids=list(range(N_CORES)),
                               trace=trace)
    out = assemble(res.results, np.asarray(w_proj, np.float32),
                   np.asarray(b_qkv, np.float32), np.asarray(b_proj, np.float32))
    return out, res


def kernel(x, w_qkv, b_qkv, w_proj, b_proj):
    out, _ = run(x, w_qkv, b_qkv, w_proj, b_proj, trace=False)
    return out


# revision 79
# speedup vs baseline: 1.0395x; 1.0395x over previous
"""Causal self-attention (B=2, T=2048, D=1024, H=16) on 8 trn2 cores.

Sharding: core c -> (batch b = c//4, head-group g = c%4, 4 heads each).
Data-parallel over B, tensor-parallel over heads; the output projection is
computed as per-core partials (each core owns 256 of the 1024 contraction
dims) summed on the host.

Per-core layout choices (all chosen so no on-device transposes are needed):
  - x is shipped feature-major x^T [D, T] (bf16, host-prepared)
  - q^T, k^T are computed feature-major [n, t] (lhsT = W^T, rhs = x^T)
  - v is computed token-major [t, e] (lhsT = x^T, rhs = W_v^T) and stored
    with a ones-column appended, so the P@V matmul emits softmax sums in
    PSUM row 64 for free
  - scores are computed transposed st[j, i] = k q^T; softmax runs without
    max-subtraction (scores are O(1) here), sums via the ones-column
  - causality: j-tile loop is bounded by the i-block, plus one triangular
    128x128 mask multiply on diagonal blocks
"""

import numpy as np
import ml_dtypes

import concourse.bass as bass
import concourse.bacc as bacc
import concourse.mybir as mybir
import concourse.tile as tile
from concourse.bass_utils import run_bass_kernel_spmd

BF16 = mybir.dt.bfloat16
F32 = mybir.dt.float32

B, T, D, H, HD = 2, 2048, 1024, 16, 64
N_CORES = 8
SCALE = HD ** -0.5  # 0.125


def _emit(nc, tc, xT_d, wqk_d, wv_d, wp_d, bqk_d, out_d):
    mult = mybir.AluOpType.mult
    Exp = mybir.ActivationFunctionType.Exp

    with (
        tc.tile_pool(name="const", bufs=1) as cpool,
        tc.tile_pool(name="work", bufs=12) as wpool,
        tc.tile_pool(name="norm", bufs=10) as npool,
        tc.tile_pool(name="stage", bufs=8) as spool,
        tc.tile_pool(name="ps_st", bufs=3, space="PSUM") as ps_st,
        tc.tile_pool(name="ps_pv", bufs=1, space="PSUM") as ps_pv,
    ):
        # ---- constants / persistent tensors -----------------------------
        mask_sb = cpool.tile([128, 128], BF16, name="mask_sb")
        nc.vector.memset(mask_sb[:], 1.0)
        # keep where (i - j) >= 0  (j = partition, i = free)
        nc.gpsimd.affine_select(
            out=mask_sb[:], in_=mask_sb[:],
            compare_op=mybir.AluOpType.is_ge, fill=0.0,
            base=0, channel_multiplier=-1, pattern=[[1, 128]],
        )

        # x^T streamed in column chunks (512 / 512 / 1024); the first wave is
        # interleaved per k-tile with the matching w_qk slab so the first
        # accumulation matmul starts after two small DMAs, not the whole load
        wqk_sb = cpool.tile([128, 8, 512], BF16, name="wqk_sb")
        wqr = wqk_d.ap().rearrange("(k p) n -> p k n", p=128)
        xt_sb = cpool.tile([128, 8, T], BF16, name="xt_sb")
        xr = xT_d.ap().rearrange("(k p) t -> p k t", p=128)
        for k in range(8):
            nc.sync.dma_start(out=wqk_sb[:, k, :], in_=wqr[:, k, :])
            nc.sync.dma_start(out=xt_sb[:, k, 0:512], in_=xr[:, k, 0:512])
        bqk_sb = cpool.tile([128, 4], F32, name="bqk_sb")
        nc.sync.dma_start(out=bqk_sb[:], in_=bqk_d[:])
        wv_sb = cpool.tile([128, 8, 256], BF16, name="wv_sb")
        nc.sync.dma_start(
            out=wv_sb[:], in_=wv_d.ap().rearrange("(k p) n -> p k n", p=128))
        for c0, c1 in ((512, 1024), (1024, 2048)):
            for k in range(8):
                nc.sync.dma_start(out=xt_sb[:, k, c0:c1],
                                  in_=xr[:, k, c0:c1])

        wp_sb = cpool.tile([128, 2, 1024], BF16, name="wp_sb")
        nc.sync.dma_start(
            out=wp_sb[:], in_=wp_d.ap().rearrange("(k p) n -> p k n", p=128))

        # sums staging for the batched reciprocal: rows 0 / 64 hold the two
        # heads' softmax sums; other rows stay 1.0 so reciprocal is finite
        srow_sb = cpool.tile([65, 512], F32, name="srow_sb")
        nc.vector.memset(srow_sb[:], 1.0)

        qkT_sb = cpool.tile([128, 4, T], BF16, name="qkT_sb")
        vaug_sb = cpool.tile([128, 4, 16, 65], BF16, name="vaug_sb")
        nc.vector.memset(vaug_sb[:], 1.0)  # ones column (col 64) survives
        outT_sb = cpool.tile([128, 2, T], BF16, name="outT_sb")

        # ---- phase A: q^T / k^T projection  (feature-major) -------------
        # m-tile: 0 = q heads 0-1, 1 = q heads 2-3, 2 = k heads 0-1, 3 = k 2-3
        def qk_unit(m, tb):
            ps = ps_st.tile([128, 512], F32, name=f"psqk{m}{tb}", tag="st")
            for k in range(8):
                nc.tensor.matmul(
                    ps[:],
                    lhsT=wqk_sb[:, k, 128 * m:128 * m + 128],
                    rhs=xt_sb[:, k, 512 * tb:512 * tb + 512],
                    start=(k == 0), stop=(k == 7),
                )
            # ScalarE eviction keeps the DVE queue (psum evictions,
            # reciprocals) from gating the shared psum slots
            nc.scalar.activation(
                out=qkT_sb[:, m, 512 * tb:512 * tb + 512],
                in_=ps[:], func=mybir.ActivationFunctionType.Identity,
                bias=bqk_sb[:, m:m + 1],
            )

        # ---- phase B: v projection (token-major, no bias) ---------------
        def emit_v(jt):
            psv = ps_st.tile([128, 256], F32, name=f"psv{jt}", tag="st")
            for k in range(8):
                nc.tensor.matmul(
                    psv[:],
                    lhsT=xt_sb[:, k, 128 * jt:128 * jt + 128],
                    rhs=wv_sb[:, k, :],
                    start=(k == 0), stop=(k == 7),
                )
            if jt < 8:
                # early v tiles: ScalarE still has slack before exp ramps,
                # and this keeps the psum-slot release off the DVE queue
                nc.scalar.copy(
                    out=vaug_sb[:, :, jt, 0:64],
                    in_=psv[:].rearrange("p (h e) -> p h e", h=4),
                )
            else:
                nc.vector.tensor_copy(
                    out=vaug_sb[:, :, jt, 0:64],
                    in_=psv[:].rearrange("p (h e) -> p h e", h=4),
                )

        # ---- phase C: attention for head pair hp (heads 2hp, 2hp+1) -----
        # Normalization (reciprocal + broadcast + scale) of block N is
        # emitted at the start of block N+1 so the expensive DVE reciprocal
        # overlaps the next block's matmuls (pv pool is double-buffered).
        pending = []

        def emit_norm(upv, hp, it):
            # both heads' sums into srow rows 0 / 64, one batched reciprocal
            # (cost scales with free size, not rows), then per-head broadcast.
            # partition_broadcast requires base partition 0, so head1's recip
            # row is first copied down to a base-0 tile.
            nc.vector.tensor_copy(out=srow_sb[0:1, :], in_=upv[0][64:65, :])
            nc.vector.tensor_copy(out=srow_sb[64:65, :], in_=upv[1][64:65, :])
            rec = npool.tile([65, 512], F32, name=f"rec{hp}{it}", tag="rec")
            nc.vector.reciprocal(out=rec[:], in_=srow_sb[:])
            recb = npool.tile([1, 512], F32, name=f"recb{hp}{it}", tag="recb")
            nc.vector.tensor_copy(out=recb[:], in_=rec[64:65, :])
            for h in range(2):
                bc = npool.tile([64, 512], F32, name=f"bc{hp}{it}{h}",
                                tag="bc")
                nc.gpsimd.partition_broadcast(
                    bc[:], rec[0:1, :] if h == 0 else recb[:])
                osl = outT_sb[64 * h:64 * h + 64, hp,
                              512 * it:512 * it + 512]
                # v-bias is folded into the host-side output bias
                nc.vector.tensor_tensor(
                    out=osl, in0=upv[h][0:64, :], in1=bc[:], op=mult)

        def emit_attn(hp, its, fillers=()):
            # fillers: small PE-only work units (qk/v/proj psum chains)
            # interleaved between attention j-tiles so the PE has independent
            # work while ScalarE exponentiates
            fillers = list(fillers)
            njt_total = sum(4 * (it + 1) for it in its)
            space = max(1, njt_total // (len(fillers) + 1)) if fillers else 0
            jt_count = 0
            for it in its:
                pv = []
                for h in range(2):
                    t_ = ps_pv.tile([65, 512], F32, name=f"pv{hp}{it}{h}",
                                    tag=f"pv{h}")
                    pv.append(t_)
                njt = 4 * (it + 1)
                for jt in range(njt):
                    diag = jt - 4 * it  # >= 0 on/near the diagonal
                    off = 128 * diag if diag >= 0 else 0
                    width = 512 - off
                    st = ps_st.tile([128, 1024], F32, name=f"st{hp}{it}{jt}",
                                    tag="st")
                    for h in range(2):
                        # explicit row-group tile_position: the two heads'
                        # K=64 matmuls run concurrently in disjoint array rows
                        nc.tensor.matmul(
                            st[:, 512 * h + off:512 * h + 512],
                            lhsT=qkT_sb[64 * h:64 * h + 64, 2 + hp,
                                        128 * jt:128 * jt + 128],
                            rhs=qkT_sb[64 * h:64 * h + 64, hp,
                                       512 * it + off:512 * it + 512],
                            start=True, stop=True,
                            tile_position=(64 * h, 0),
                        )
                    ex = wpool.tile([128, 1024], BF16, name=f"ex{hp}{it}{jt}",
                                    tag="ex")
                    st_v = st[:].rearrange("p (h i) -> p h i", h=2)
                    ex_v = ex[:].rearrange("p (h i) -> p h i", h=2)
                    nc.scalar.activation(
                        out=ex_v[:, :, off:512], in_=st_v[:, :, off:512],
                        func=Exp, scale=SCALE,
                    )
                    if diag >= 0:
                        for h in range(2):
                            sl = ex[:, 512 * h + off:512 * h + off + 128]
                            nc.vector.tensor_tensor(
                                out=sl, in0=sl, in1=mask_sb[:], op=mult)
                    for h in range(2):
                        nc.tensor.matmul(
                            pv[h][:, off:512],
                            lhsT=vaug_sb[:, 2 * hp + h, jt, :],
                            rhs=ex[:, 512 * h + off:512 * h + 512],
                            start=(jt == 0), stop=(jt == njt - 1),
                        )
                    jt_count += 1
                    if jt == min(1, njt - 1) and pending:
                        # normalize the previous block mid-loop: the first
                        # mask-multiplies of this block are already queued on
                        # DVE ahead of the expensive reciprocal
                        emit_norm(*pending.pop(0))
                    if fillers and jt_count % space == 0:
                        fillers.pop(0)()
                # evict the PV accumulators (+ sums row) to SBUF right away:
                # frees the psum banks after one quick DVE copy instead of
                # after the whole normalization chain
                upv = []
                for h in range(2):
                    u = npool.tile([65, 512], F32, name=f"upv{hp}{it}{h}",
                                   tag=f"upv{h}", bufs=2)
                    nc.vector.tensor_copy(out=u[:], in_=pv[h][:])
                    upv.append(u)
                pending.append((upv, hp, it))
            for f in fillers:
                f()

        # ---- phase D: output projection partials (bf16, summed on host) --
        def proj_unit(tt):
            for ch in range(2):
                psp = ps_st.tile([128, 512], F32, name=f"psp{tt}{ch}",
                                 tag="st")
                for ek in range(2):
                    nc.tensor.matmul(
                        psp[:],
                        lhsT=outT_sb[:, ek, 128 * tt:128 * tt + 128],
                        rhs=wp_sb[:, ek, 512 * ch:512 * ch + 512],
                        start=(ek == 0), stop=(ek == 1),
                    )
                stg = spool.tile([128, 512], BF16, name=f"stg{tt}{ch}",
                                 tag="stage")
                if tt >= 12 and ch == 0:
                    # tail blocks: DVE is idle here — split the eviction
                    # queues so neither engine serializes the finish
                    nc.vector.tensor_copy(out=stg[:], in_=psp[:])
                else:
                    # ScalarE copy: keeps psum eviction off the DVE queue
                    nc.scalar.copy(out=stg[:], in_=psp[:])
                nc.sync.dma_start(
                    out=out_d.ap()[128 * tt:128 * tt + 128,
                                   512 * ch:512 * ch + 512],
                    in_=stg[:])

        # Interleaved emission: attention (exp on ScalarE is the long pole)
        # starts as soon as its first q/k/v tiles are projected; remaining
        # projection work is spread through the attention loops as fillers.
        qk_unit(0, 0)
        qk_unit(2, 0)
        for jt in range(4):
            emit_v(jt)
        emit_attn(0, [0])
        qk_unit(0, 1)
        qk_unit(2, 1)
        for jt in range(4, 8):
            emit_v(jt)
        emit_attn(0, [1])
        for tb in range(2, 4):
            qk_unit(0, tb)
            qk_unit(2, tb)
        for jt in range(8, 16):
            emit_v(jt)
        emit_attn(0, [2, 3])
        for tb in range(2):
            qk_unit(1, tb)
            qk_unit(3, tb)
        emit_attn(1, [0, 1])
        for tb in range(2, 4):
            qk_unit(1, tb)
            qk_unit(3, tb)
        for tt in range(4):              # t-block 0 (norms hp0/hp1 done)
            proj_unit(tt)
        emit_attn(1, [2, 3])
        emit_norm(*pending.pop(0))       # last block's normalization
        for tt in range(4, 16):
            proj_unit(tt)


def build_program():
    nc = bacc.Bacc("TRN2", target_bir_lowering=False, debug=False,
                   num_devices=N_CORES)
    xT_d = nc.dram_tensor("xT", [D, T], BF16, kind="ExternalInput")
    wqk_d = nc.dram_tensor("wqk", [D, 512], BF16, kind="ExternalInput")
    wv_d = nc.dram_tensor("wv", [D, 256], BF16, kind="ExternalInput")
    wp_d = nc.dram_tensor("wp", [256, D], BF16, kind="ExternalInput")
    bqk_d = nc.dram_tensor("bqk", [128, 4], F32, kind="ExternalInput")
    out_d = nc.dram_tensor("out", [T, D], BF16, kind="ExternalOutput")
    with tile.TileContext(nc) as tc:
        _emit(nc, tc, xT_d, wqk_d, wv_d, wp_d, bqk_d, out_d)
    nc.compile()
    return nc


def make_in_maps(x, w_qkv, b_qkv, w_proj):
    bf = ml_dtypes.bfloat16
    in_maps = []
    for c in range(N_CORES):
        b, g = c // 4, c % 4
        qs, ks, vs = 256 * g, 1024 + 256 * g, 2048 + 256 * g
        qb = b_qkv[qs:qs + 256]
        kb = b_qkv[ks:ks + 256]
        vb = b_qkv[vs:vs + 256]
        in_maps.append({
            "xT": np.ascontiguousarray(x[b].T).astype(bf),
            "wqk": np.ascontiguousarray(
                np.concatenate([w_qkv[qs:qs + 256], w_qkv[ks:ks + 256]], 0).T
            ).astype(bf),
            "wv": np.ascontiguousarray(w_qkv[vs:vs + 256].T).astype(bf),
            "wp": np.ascontiguousarray(w_proj[:, 256 * g:256 * g + 256].T
                                       ).astype(bf),
            "bqk": np.ascontiguousarray(
                np.stack([qb[:128], qb[128:], kb[:128], kb[128:]], 1)
            ).astype(np.float32),
        })
    return in_maps


def assemble(results, w_proj, b_qkv, b_proj):
    out = np.zeros((B, T, D), np.float32)
    for c in range(N_CORES):
        out[c // 4] += results[c]["out"].astype(np.float32)
    # v-bias passes through attention unchanged (rows of P sum to 1), so its
    # contribution to the output is the constant w_proj @ b_v
    bias = b_proj.astype(np.float32) + w_proj.astype(np.float32) @ \
        b_qkv[2048:3072].astype(np.float32)
    out += bias[None, None, :]
    return out


_NC_CACHE = []


def run(x, w_qkv, b_qkv, w_proj, b_proj, trace=False):
    if not _NC_CACHE:
        _NC_CACHE.append(build_program())
    nc = _NC_CACHE[0]
    in_maps = make_in_maps(np.asarray(x, np.float32), np.asarray(w_qkv, np.float32),
                           np.asarray(b_qkv, np.float32), np.asarray(w_proj, np.float32))
    res = run_bass_kernel_spmd(nc, in_maps, core_ids=list(range(N_CORES)),
                               trace=trace)
    out = assemble(res.results, np.asarray(w_proj, np.float32),
                   np.asarray(b_qkv, np.float32), np.asarray(b_proj, np.float32))
    return out, res


def kernel(x, w_qkv, b_qkv, w_proj, b_proj):
    out, _ = run(x, w_qkv, b_qkv, w_proj, b_proj, trace=False)
    return out


# revision 80
# speedup vs baseline: 1.0585x; 1.0183x over previous
"""Causal self-attention (B=2, T=2048, D=1024, H=16) on 8 trn2 cores.

Sharding: core c -> (batch b = c//4, head-group g = c%4, 4 heads each).
Data-parallel over B, tensor-parallel over heads; the output projection is
computed as per-core partials (each core owns 256 of the 1024 contraction
dims) summed on the host.

Per-core layout choices (all chosen so no on-device transposes are needed):
  - x is shipped feature-major x^T [D, T] (bf16, host-prepared)
  - q^T, k^T are computed feature-major [n, t] (lhsT = W^T, rhs = x^T)
  - v is computed token-major [t, e] (lhsT = x^T, rhs = W_v^T) and stored
    with a ones-column appended, so the P@V matmul emits softmax sums in
    PSUM row 64 for free
  - scores are computed transposed st[j, i] = k q^T; softmax runs without
    max-subtraction (scores are O(1) here), sums via the ones-column
  - causality: j-tile loop is bounded by the i-block, plus one triangular
    128x128 mask multiply on diagonal blocks
"""

import numpy as np
import ml_dtypes

import concourse.bass as bass
import concourse.bacc as bacc
import concourse.mybir as mybir
import concourse.tile as tile
from concourse.bass_utils import run_bass_kernel_spmd

BF16 = mybir.dt.bfloat16
F32 = mybir.dt.float32

B, T, D, H, HD = 2, 2048, 1024, 16, 64
N_CORES = 8
SCALE = HD ** -0.5  # 0.125


def _emit(nc, tc, xT_d, wqk_d, wv_d, wp_d, bqk_d, out_d):
    mult = mybir.AluOpType.mult
    Exp = mybir.ActivationFunctionType.Exp

    with (
        tc.tile_pool(name="const", bufs=1) as cpool,
        tc.tile_pool(name="work", bufs=12) as wpool,
        tc.tile_pool(name="norm", bufs=10) as npool,
        tc.tile_pool(name="stage", bufs=8) as spool,
        tc.tile_pool(name="ps_st", bufs=3, space="PSUM") as ps_st,
        tc.tile_pool(name="ps_pv", bufs=1, space="PSUM") as ps_pv,
    ):
        # ---- constants / persistent tensors -----------------------------
        mask_sb = cpool.tile([128, 128], BF16, name="mask_sb")
        nc.vector.memset(mask_sb[:], 1.0)
        # keep where (i - j) >= 0  (j = partition, i = free)
        nc.gpsimd.affine_select(
            out=mask_sb[:], in_=mask_sb[:],
            compare_op=mybir.AluOpType.is_ge, fill=0.0,
            base=0, channel_multiplier=-1, pattern=[[1, 128]],
        )

        # x^T streamed in column chunks (512 / 512 / 1024); the first wave is
        # interleaved per k-tile with the matching w_qk slab so the first
        # accumulation matmul starts after two small DMAs, not the whole load
        wqk_sb = cpool.tile([128, 8, 512], BF16, name="wqk_sb")
        wqr = wqk_d.ap().rearrange("(k p) n -> p k n", p=128)
        xt_sb = cpool.tile([128, 8, T], BF16, name="xt_sb")
        xr = xT_d.ap().rearrange("(k p) t -> p k t", p=128)
        for k in range(8):
            nc.sync.dma_start(out=wqk_sb[:, k, :], in_=wqr[:, k, :])
            nc.sync.dma_start(out=xt_sb[:, k, 0:512], in_=xr[:, k, 0:512])
        bqk_sb = cpool.tile([128, 4], F32, name="bqk_sb")
        nc.sync.dma_start(out=bqk_sb[:], in_=bqk_d[:])
        wv_sb = cpool.tile([128, 8, 256], BF16, name="wv_sb")
        nc.sync.dma_start(
            out=wv_sb[:], in_=wv_d.ap().rearrange("(k p) n -> p k n", p=128))
        for c0, c1 in ((512, 1024), (1024, 2048)):
            for k in range(8):
                nc.sync.dma_start(out=xt_sb[:, k, c0:c1],
                                  in_=xr[:, k, c0:c1])

        wp_sb = cpool.tile([128, 2, 1024], BF16, name="wp_sb")
        nc.sync.dma_start(
            out=wp_sb[:], in_=wp_d.ap().rearrange("(k p) n -> p k n", p=128))

        # sums staging for the batched reciprocal: rows 0 / 64 hold the two
        # heads' softmax sums; other rows stay 1.0 so reciprocal is finite
        srow_sb = cpool.tile([65, 512], F32, name="srow_sb")
        nc.vector.memset(srow_sb[:], 1.0)

        qkT_sb = cpool.tile([128, 4, T], BF16, name="qkT_sb")
        vaug_sb = cpool.tile([128, 4, 16, 65], BF16, name="vaug_sb")
        nc.vector.memset(vaug_sb[:], 1.0)  # ones column (col 64) survives
        outT_sb = cpool.tile([128, 2, T], BF16, name="outT_sb")

        # ---- phase A: q^T / k^T projection  (feature-major) -------------
        # m-tile: 0 = q heads 0-1, 1 = q heads 2-3, 2 = k heads 0-1, 3 = k 2-3
        def qk_unit(m, tb):
            ps = ps_st.tile([128, 512], F32, name=f"psqk{m}{tb}", tag="st")
            for k in range(8):
                nc.tensor.matmul(
                    ps[:],
                    lhsT=wqk_sb[:, k, 128 * m:128 * m + 128],
                    rhs=xt_sb[:, k, 512 * tb:512 * tb + 512],
                    start=(k == 0), stop=(k == 7),
                )
            # ScalarE eviction keeps the DVE queue (psum evictions,
            # reciprocals) from gating the shared psum slots
            nc.scalar.activation(
                out=qkT_sb[:, m, 512 * tb:512 * tb + 512],
                in_=ps[:], func=mybir.ActivationFunctionType.Identity,
                bias=bqk_sb[:, m:m + 1],
            )

        # ---- phase B: v projection (token-major, no bias) ---------------
        def emit_v(jt):
            psv = ps_st.tile([128, 256], F32, name=f"psv{jt}", tag="st")
            for k in range(8):
                nc.tensor.matmul(
                    psv[:],
                    lhsT=xt_sb[:, k, 128 * jt:128 * jt + 128],
                    rhs=wv_sb[:, k, :],
                    start=(k == 0), stop=(k == 7),
                )
            if jt < 16:
                # early v tiles: ScalarE still has slack before exp ramps,
                # and this keeps the psum-slot release off the DVE queue
                nc.scalar.copy(
                    out=vaug_sb[:, :, jt, 0:64],
                    in_=psv[:].rearrange("p (h e) -> p h e", h=4),
                )
            else:
                nc.vector.tensor_copy(
                    out=vaug_sb[:, :, jt, 0:64],
                    in_=psv[:].rearrange("p (h e) -> p h e", h=4),
                )

        # ---- phase C: attention for head pair hp (heads 2hp, 2hp+1) -----
        # Normalization (reciprocal + broadcast + scale) of block N is
        # emitted at the start of block N+1 so the expensive DVE reciprocal
        # overlaps the next block's matmuls (pv pool is double-buffered).
        pending = []

        def emit_norm(upv, hp, it):
            # both heads' sums into srow rows 0 / 64, one batched reciprocal
            # (cost scales with free size, not rows), then per-head broadcast.
            # partition_broadcast requires base partition 0, so head1's recip
            # row is first copied down to a base-0 tile.
            nc.vector.tensor_copy(out=srow_sb[0:1, :], in_=upv[0][64:65, :])
            nc.vector.tensor_copy(out=srow_sb[64:65, :], in_=upv[1][64:65, :])
            rec = npool.tile([65, 512], F32, name=f"rec{hp}{it}", tag="rec")
            nc.vector.reciprocal(out=rec[:], in_=srow_sb[:])
            recb = npool.tile([1, 512], F32, name=f"recb{hp}{it}", tag="recb")
            nc.vector.tensor_copy(out=recb[:], in_=rec[64:65, :])
            for h in range(2):
                bc = npool.tile([64, 512], F32, name=f"bc{hp}{it}{h}",
                                tag="bc")
                nc.gpsimd.partition_broadcast(
                    bc[:], rec[0:1, :] if h == 0 else recb[:])
                osl = outT_sb[64 * h:64 * h + 64, hp,
                              512 * it:512 * it + 512]
                # v-bias is folded into the host-side output bias
                nc.vector.tensor_tensor(
                    out=osl, in0=upv[h][0:64, :], in1=bc[:], op=mult)

        def emit_attn(hp, its, fillers=()):
            # fillers: small PE-only work units (qk/v/proj psum chains)
            # interleaved between attention j-tiles so the PE has independent
            # work while ScalarE exponentiates
            fillers = list(fillers)
            njt_total = sum(4 * (it + 1) for it in its)
            space = max(1, njt_total // (len(fillers) + 1)) if fillers else 0
            jt_count = 0
            for it in its:
                pv = []
                for h in range(2):
                    t_ = ps_pv.tile([65, 512], F32, name=f"pv{hp}{it}{h}",
                                    tag=f"pv{h}")
                    pv.append(t_)
                njt = 4 * (it + 1)
                for jt in range(njt):
                    diag = jt - 4 * it  # >= 0 on/near the diagonal
                    off = 128 * diag if diag >= 0 else 0
                    width = 512 - off
                    st = ps_st.tile([128, 1024], F32, name=f"st{hp}{it}{jt}",
                                    tag="st")
                    for h in range(2):
                        # explicit row-group tile_position: the two heads'
                        # K=64 matmuls run concurrently in disjoint array rows
                        nc.tensor.matmul(
                            st[:, 512 * h + off:512 * h + 512],
                            lhsT=qkT_sb[64 * h:64 * h + 64, 2 + hp,
                                        128 * jt:128 * jt + 128],
                            rhs=qkT_sb[64 * h:64 * h + 64, hp,
                                       512 * it + off:512 * it + 512],
                            start=True, stop=True,
                            tile_position=(64 * h, 0),
                        )
                    ex = wpool.tile([128, 1024], BF16, name=f"ex{hp}{it}{jt}",
                                    tag="ex")
                    st_v = st[:].rearrange("p (h i) -> p h i", h=2)
                    ex_v = ex[:].rearrange("p (h i) -> p h i", h=2)
                    nc.scalar.activation(
                        out=ex_v[:, :, off:512], in_=st_v[:, :, off:512],
                        func=Exp, scale=SCALE,
                    )
                    if diag >= 0:
                        for h in range(2):
                            sl = ex[:, 512 * h + off:512 * h + off + 128]
                            nc.vector.tensor_tensor(
                                out=sl, in0=sl, in1=mask_sb[:], op=mult)
                    for h in range(2):
                        nc.tensor.matmul(
                            pv[h][:, off:512],
                            lhsT=vaug_sb[:, 2 * hp + h, jt, :],
                            rhs=ex[:, 512 * h + off:512 * h + 512],
                            start=(jt == 0), stop=(jt == njt - 1),
                        )
                    jt_count += 1
                    if jt == min(1, njt - 1) and pending:
                        # normalize the previous block mid-loop: the first
                        # mask-multiplies of this block are already queued on
                        # DVE ahead of the expensive reciprocal
                        emit_norm(*pending.pop(0))
                    if fillers and jt_count % space == 0:
                        fillers.pop(0)()
                # evict the PV accumulators (+ sums row) to SBUF right away:
                # frees the psum banks after one quick DVE copy instead of
                # after the whole normalization chain
                upv = []
                for h in range(2):
                    u = npool.tile([65, 512], F32, name=f"upv{hp}{it}{h}",
                                   tag=f"upv{h}", bufs=2)
                    nc.vector.tensor_copy(out=u[:], in_=pv[h][:])
                    upv.append(u)
                pending.append((upv, hp, it))
            for f in fillers:
                f()

        # ---- phase D: output projection partials (bf16, summed on host) --
        def proj_unit(tt):
            for ch in range(2):
                psp = ps_st.tile([128, 512], F32, name=f"psp{tt}{ch}",
                                 tag="st")
                for ek in range(2):
                    nc.tensor.matmul(
                        psp[:],
                        lhsT=outT_sb[:, ek, 128 * tt:128 * tt + 128],
                        rhs=wp_sb[:, ek, 512 * ch:512 * ch + 512],
                        start=(ek == 0), stop=(ek == 1),
                    )
                stg = spool.tile([128, 512], BF16, name=f"stg{tt}{ch}",
                                 tag="stage")
                if tt >= 12 and ch == 0:
                    # tail blocks: DVE is idle here — split the eviction
                    # queues so neither engine serializes the finish
                    nc.vector.tensor_copy(out=stg[:], in_=psp[:])
                else:
                    # ScalarE copy: keeps psum eviction off the DVE queue
                    nc.scalar.copy(out=stg[:], in_=psp[:])
                nc.sync.dma_start(
                    out=out_d.ap()[128 * tt:128 * tt + 128,
                                   512 * ch:512 * ch + 512],
                    in_=stg[:])

        # Interleaved emission: attention (exp on ScalarE is the long pole)
        # starts as soon as its first q/k/v tiles are projected; remaining
        # projection work is spread through the attention loops as fillers.
        qk_unit(0, 0)
        qk_unit(2, 0)
        for jt in range(4):
            emit_v(jt)
        emit_attn(0, [0])
        qk_unit(0, 1)
        qk_unit(2, 1)
        for jt in range(4, 8):
            emit_v(jt)
        emit_attn(0, [1])
        for tb in range(2, 4):
            qk_unit(0, tb)
            qk_unit(2, tb)
        for jt in range(8, 16):
            emit_v(jt)
        emit_attn(0, [2, 3])
        for tb in range(2):
            qk_unit(1, tb)
            qk_unit(3, tb)
        emit_attn(1, [0, 1])
        for tb in range(2, 4):
            qk_unit(1, tb)
            qk_unit(3, tb)
        for tt in range(4):              # t-block 0 (norms hp0/hp1 done)
            proj_unit(tt)
        emit_attn(1, [2, 3])
        emit_norm(*pending.pop(0))       # last block's normalization
        for tt in range(4, 16):
            proj_unit(tt)


def build_program():
    nc = bacc.Bacc("TRN2", target_bir_lowering=False, debug=False,
                   num_devices=N_CORES)
    xT_d = nc.dram_tensor("xT", [D, T], BF16, kind="ExternalInput")
    wqk_d = nc.dram_tensor("wqk", [D, 512], BF16, kind="ExternalInput")
    wv_d = nc.dram_tensor("wv", [D, 256], BF16, kind="ExternalInput")
    wp_d = nc.dram_tensor("wp", [256, D], BF16, kind="ExternalInput")
    bqk_d = nc.dram_tensor("bqk", [128, 4], F32, kind="ExternalInput")
    out_d = nc.dram_tensor("out", [T, D], BF16, kind="ExternalOutput")
    with tile.TileContext(nc) as tc:
        _emit(nc, tc, xT_d, wqk_d, wv_d, wp_d, bqk_d, out_d)
    nc.compile()
    return nc


def make_in_maps(x, w_qkv, b_qkv, w_proj):
    bf = ml_dtypes.bfloat16
    in_maps = []
    for c in range(N_CORES):
        b, g = c // 4, c % 4
        qs, ks, vs = 256 * g, 1024 + 256 * g, 2048 + 256 * g
        qb = b_qkv[qs:qs + 256]
        kb = b_qkv[ks:ks + 256]
        vb = b_qkv[vs:vs + 256]
        in_maps.append({
            "xT": np.ascontiguousarray(x[b].T).astype(bf),
            "wqk": np.ascontiguousarray(
                np.concatenate([w_qkv[qs:qs + 256], w_qkv[ks:ks + 256]], 0).T
            ).astype(bf),
            "wv": np.ascontiguousarray(w_qkv[vs:vs + 256].T).astype(bf),
            "wp": np.ascontiguousarray(w_proj[:, 256 * g:256 * g + 256].T
                                       ).astype(bf),
            "bqk": np.ascontiguousarray(
                np.stack([qb[:128], qb[128:], kb[:128], kb[128:]], 1)
            ).astype(np.float32),
        })
    return in_maps


def assemble(results, w_proj, b_qkv, b_proj):
    out = np.zeros((B, T, D), np.float32)
    for c in range(N_CORES):
        out[c // 4] += results[c]["out"].astype(np.float32)
    # v-bias passes through attention unchanged (rows of P sum to 1), so its
    # contribution to the output is the constant w_proj @ b_v
    bias = b_proj.astype(np.float32) + w_proj.astype(np.float32) @ \
        b_qkv[2048:3072].astype(np.float32)
    out += bias[None, None, :]
    return out


_NC_CACHE = []


def run(x, w_qkv, b_qkv, w_proj, b_proj, trace=False):
    if not _NC_CACHE:
        _NC_CACHE.append(build_program())
    nc = _NC_CACHE[0]
    in_maps = make_in_maps(np.asarray(x, np.float32), np.asarray(w_qkv, np.float32),
                           np.asarray(b_qkv, np.float32), np.asarray(w_proj, np.float32))
    res = run_bass_kernel_spmd(nc, in_maps, core_ids=list(range(N_CORES)),
                               trace=trace)
    out = assemble(res.results, np.asarray(w_proj, np.float32),
                   np.asarray(b_qkv, np.float32), np.asarray(b_proj, np.float32))
    return out, res


def kernel(x, w_qkv, b_qkv, w_proj, b_proj):
    out, _ = run(x, w_qkv, b_qkv, w_proj, b_proj, trace=False)
    return out
